# revision 24
# baseline (speedup 1.0000x reference)
"""Trainium2 Bass kernel for nn_Attention_50345606644062 (sparse_attention).

Architecture (per reference):
  kv = dwconv3x3_grouped(pwconv(x, kv_w), kv_dw_w); k, v = split(kv)
  q  = conv3x3_dense(pwconv(y, q_w), q_dw_w)
  q,k l2-normalized along spatial; attn = softmax(q_n @ k_n^T * temp) per head
  out = proj_pw(attn @ v)

Strategy: data-parallel over batch (8 cores, 1 image each). All matmuls bf16
with f32 PSUM accumulation. L2 norms come free from Gram-matrix diagonals
(q/k Grams accumulated over DMA-transposed spatial chunks). softmax + proj
fold into one fused [192,192] weight so attn@v + proj = a single dense matmul.
Spatial processed in 4 row-bands of 32 to fit SBUF.
"""
import sys
sys.path.insert(0, '/opt/trn_rl_repo')

import numpy as np
import ml_dtypes
from contextlib import ExitStack

import concourse.bass as bass
import concourse.tile as tile
from concourse import bacc, mybir
from concourse import bass_utils

BF = mybir.dt.bfloat16
F32 = mybir.dt.float32
AF = mybir.ActivationFunctionType

B, C, HEADS, H, W = 8, 192, 8, 128, 128
HD = C // HEADS          # 24
HW = H * W               # 16384
RT = 4                   # output rows per e-tile
NT = RT * W              # 512 free elems per e-tile
NTILES = H // RT         # 32
NBANDS = 4
BROWS = H // NBANDS      # 32 rows per band
BTILES = BROWS // RT     # 8 e-tiles per band
PB = BROWS + 2           # padded rows per band buffer
EPS = 1e-12

CBQ = [(0, 128), (128, 64)]   # q-path channel blocks
CB96 = [(0, 96), (96, 96)]    # kv-path channel blocks

_NC_CACHE = {}


_copy_rr = [0]

def _copy(nc, eng_idx, out, in_):
    """Alternate psum->sbuf copies between DVE and ACT."""
    _copy_rr[0] += 1
    if _copy_rr[0] % 3 == 0:
        nc.vector.tensor_copy(out, in_)
    else:
        nc.scalar.copy(out, in_)


def _conv_taps(nc, ps_v, lhsT_fn, src, r0, kblocks):
    """Accumulate 3x3 conv taps into psum view ps_v [P, RT, W].
    src(k): padded band AP viewed [P, rows, W]. lhsT_fn(dy, dx, k)."""
    first = True
    for dy in range(3):
        for dx in (1, 0, 2):  # full-width tap first (sets has_written everywhere)
            for k in range(kblocks):
                lhsT = lhsT_fn(dy, dx, k)
                s = src(k)
                if dx == 1:
                    oap, iap = ps_v[:, :, :], s[:, r0 + dy:r0 + dy + RT, :]
                elif dx == 0:
                    oap = ps_v[:, :, 1:128]
                    iap = s[:, r0 + dy:r0 + dy + RT, 0:127]
                else:
                    oap = ps_v[:, :, 0:127]
                    iap = s[:, r0 + dy:r0 + dy + RT, 1:128]
                nc.tensor.matmul(oap, lhsT, iap, start=first, stop=False,
                                 skip_group_check=True)
                first = False


def _dw_taps_vec(nc, eng, dst, ws, src, r0, acc_pool, unit):
    """Depthwise 3x3 on DVE/GPSIMD: in-place FMA chain into f32 acc, last tap
    writes dst (bf16). eng: nc.vector or nc.gpsimd. ws(t) -> [96,1] scalar AP.
    src: padded band AP viewed [96, rows, W]."""
    import concourse.mybir as mybir
    acc = acc_pool.tile([96, RT * W], mybir.dt.float32, tag="dwacc",
                        name=f"dwacc_{unit}", bufs=3)
    acc_v = acc[:].rearrange("p (r c) -> p r c", r=RT)
    taps = [(dy, dx) for dy in range(3) for dx in (1, 0, 2)]
    # first tap (full width) initializes acc; middle taps in-place; last writes dst
    for i, (dy, dx) in enumerate(taps):
        w = ws(dy * 3 + dx)
        if dx == 1:
            oap, iap = acc_v[:, :, :], src[:, r0 + dy:r0 + dy + RT, :]
        elif dx == 0:
            oap = acc_v[:, :, 1:128]
            iap = src[:, r0 + dy:r0 + dy + RT, 0:127]
        else:
            oap = acc_v[:, :, 0:127]
            iap = src[:, r0 + dy:r0 + dy + RT, 1:128]
        if i == 0:
            eng.tensor_scalar_mul(oap, iap, w)
        else:
            eng.scalar_tensor_tensor(oap, iap, w, oap,
                                     op0=mybir.AluOpType.mult,
                                     op1=mybir.AluOpType.add)
    nc.vector.tensor_copy(dst, acc[:])


def _dw_taps_actpool(nc, dst, ws, src, r0, tmp_pool, unit):
    """Depthwise 3x3 split: ACT makes scaled taps (per-partition scale AP),
    Pool accumulates in-place into an f32 acc; ACT casts acc to dst."""
    import concourse.mybir as mybir
    acc = tmp_pool.tile([96, RT * W], mybir.dt.float32, tag="dwacc",
                        name=f"dwaccp_{unit}", bufs=3)
    acc_v = acc[:].rearrange("p (r c) -> p r c", r=RT)
    taps = [(dy, dx) for dy in range(3) for dx in (1, 0, 2)]
    for i, (dy, dx) in enumerate(taps):
        w = ws(dy * 3 + dx)
        if dx == 1:
            oap = acc_v[:, :, :]
            iap = src[:, r0 + dy:r0 + dy + RT, :]
        elif dx == 0:
            oap = acc_v[:, :, 1:128]
            iap = src[:, r0 + dy:r0 + dy + RT, 0:127]
        else:
            oap = acc_v[:, :, 0:127]
            iap = src[:, r0 + dy:r0 + dy + RT, 1:128]
        if i == 0:
            nc.scalar.activation(oap, iap, AF.Copy, scale=w)
        else:
            t = tmp_pool.tile([96, RT * W], mybir.dt.bfloat16, tag="dwtmp",
                              name=f"dwtmp_{unit}_{i}", bufs=3)
            t_v = t[:].rearrange("p (r c) -> p r c", r=RT)
            tap_o = t_v[:, :, 1:128] if dx == 0 else (
                t_v[:, :, 0:127] if dx == 2 else t_v[:, :, :])
            nc.scalar.activation(tap_o, iap, AF.Copy, scale=w)
            nc.gpsimd.tensor_tensor(oap, oap, tap_o, op=mybir.AluOpType.add)
    nc.vector.tensor_copy(dst, acc[:])


def build_bass():
    if 'nc' in _NC_CACHE:
        return _NC_CACHE['nc']
    nc = bacc.Bacc("TRN2", target_bir_lowering=False, debug=False, num_devices=8)

    dram = {}
    def din(name, shape, dt):
        dram[name] = nc.dram_tensor(name, shape, dt, kind="ExternalInput").ap()
    din("xb", [C, HW], F32)
    din("yb", [C, HW], F32)
    din("qwT", [C, C], BF)
    din("w3T", [C, 9 * C], BF)
    din("kvwT", [C, 2 * C], BF)
    din("dwdK", [96, 2 * 9 * 96], BF)
    din("dwdV", [96, 2 * 9 * 96], BF)
    din("dwsK", [96, 2 * 9], F32)
    din("dwsV", [96, 2 * 9], F32)
    din("projT", [C, C], BF)
    din("temp96", [96, 2], F32)
    din("eye96", [96, 96], F32)
    din("amask", [96, 96], F32)
    out_ap = nc.dram_tensor("outb", [C, HW], F32, kind="ExternalOutput").ap()

    with tile.TileContext(nc) as tc:
        with ExitStack() as ctx:
            wpool = ctx.enter_context(tc.tile_pool(name="w", bufs=1))
            big = ctx.enter_context(tc.tile_pool(name="big", bufs=1))
            band_p = ctx.enter_context(tc.tile_pool(name="band", bufs=1))
            stg = ctx.enter_context(tc.tile_pool(name="stg", bufs=2))
            sm = ctx.enter_context(tc.tile_pool(name="sm", bufs=1))
            tpose = ctx.enter_context(tc.tile_pool(name="tpose", bufs=2))
            psw = ctx.enter_context(tc.tile_pool(name="psw", bufs=2, space="PSUM"))
            psg = ctx.enter_context(tc.tile_pool(name="psg", bufs=1, space="PSUM"))

            # ---- weights ----
            def wload(name, p0, pn, cols, dt):
                t = wpool.tile([pn, cols], dt, tag=f"{name}{p0}", name=f"{name}{p0}")
                nc.sync.dma_start(t[:], dram[name][p0:p0 + pn, :])
                return t
            qwT = [wload("qwT", p0, pn, C, BF) for p0, pn in CBQ]
            w3T = [wload("w3T", p0, pn, 9 * C, BF) for p0, pn in CBQ]
            kvwT = [wload("kvwT", p0, pn, 2 * C, BF) for p0, pn in CBQ]
            dwdK = wload("dwdK", 0, 96, 2 * 9 * 96, BF)
            dwdV = wload("dwdV", 0, 96, 2 * 9 * 96, BF)
            dwsK = wload("dwsK", 0, 96, 2 * 9, F32)
            dwsV = wload("dwsV", 0, 96, 2 * 9, F32)
            projT = [wload("projT", p0, pn, C, BF) for p0, pn in CB96]
            temp96 = wload("temp96", 0, 96, 2, F32)
            eye96 = wload("eye96", 0, 96, 96, F32)
            amask = wload("amask", 0, 96, 96, F32)

            # ---- persistent: v ----
            vt = [big.tile([96, HW], BF, tag=f"v{j}", name=f"v{j}") for j in range(2)]

            # ---- gram psum (accumulates across all bands) ----
            gbank0 = psg.tile([96, 480], F32, tag="g0", name="g0")
            gbank1 = psg.tile([96, 96], F32, tag="g1", name="g1")
            def gram_slot(kind, g):
                idx = {("qq", 0): 0, ("qk", 0): 1, ("kk", 0): 2,
                       ("qq", 1): 3, ("qk", 1): 4}.get((kind, g))
                return gbank1[:, :] if idx is None else gbank0[:, idx * 96:(idx + 1) * 96]

            def pw_band(band, src_name, wts, ocs, dsts, stag):
                """Pointwise conv for band rows (with 1-row halo each side)."""
                cast_cache = {}
                def get_cast(tt):
                    # load+cast in 1024-wide pairs (tt0 even-aligned pair)
                    tt0 = tt - (tt % 2)
                    if tt0 not in cast_cache:
                        wide = min(2, NTILES - tt0)
                        res = []
                        for i, (p0, pn) in enumerate(CBQ):
                            st = stg.tile([pn, wide * NT], F32, tag=f"st{i}",
                                          name=f"{stag}s{i}_{band}_{tt0}")
                            nc.sync.dma_start(
                                st[:], dram[src_name][p0:p0 + pn,
                                                      bass.ds(tt0 * NT, wide * NT)])
                            xb = stg.tile([pn, wide * NT], BF, tag=f"bf{i}",
                                          name=f"{stag}b{i}_{band}_{tt0}")
                            nc.gpsimd.tensor_copy(xb[:], st[:])
                            res.append(xb)
                        cast_cache[tt0] = res
                    return cast_cache[tt0], tt - tt0
                for tt in range(BTILES * band - 1, BTILES * band + BTILES + 1):
                    if tt < 0 or tt >= NTILES:
                        continue
                    wtiles, off = get_cast(tt)
                    xbf = [w[:, off * NT:(off + 1) * NT] for w in wtiles]
                    pr = RT * tt - BROWS * band + 1
                    lo = max(0, -pr); hi = min(RT, PB - pr)
                    for ci, (o0, on, di) in enumerate(ocs):
                        ptag = "psq" if stag == "y" else ("psk" if di < 2 else "psv")
                        ps = psw.tile([on, NT], F32, tag=ptag,
                                      name=f"{stag}ps{ci}_{band}_{tt}")
                        for ki in range(2):
                            nc.tensor.matmul(ps[:], wts[ki][:, o0:o0 + on], xbf[ki],
                                             start=(ki == 0), stop=(ki == 1))
                        dst = dsts[di][:, (pr + lo) * W:(pr + hi) * W]
                        _copy(nc, (ci + di) % 2, dst, ps[:, lo * W:hi * W])

            def zero_halos(band, tiles):
                if band == 0:
                    for t in tiles:
                        nc.gpsimd.memset(t[:, 0:W], 0.0)
                if band == NBANDS - 1:
                    for t in tiles:
                        nc.gpsimd.memset(t[:, (PB - 1) * W:], 0.0)


            dw_unit = [0]
            def dw_engine(band, is_k, tt=0, j=0):
                if is_k:
                    return "PE"
                if band == NBANDS - 1 and tt >= BTILES - 2:
                    return "PE"  # tail tiles on PE so final matmuls aren't starved
                return "DVE"
            chunk_no = [0]
            n_chunk_total = NTILES * RT

            for band in range(NBANDS):
                # --- q path: pointwise -> padded band buffer ---
                q1p = [band_p.tile([pn, PB * W], BF, tag=f"q1p{i}",
                                   name=f"q1p{i}_{band}")
                       for i, (p0, pn) in enumerate(CBQ)]
                pw_band(band, "yb", qwT,
                        [(p0, pn, i) for i, (p0, pn) in enumerate(CBQ)], q1p, "y")
                zero_halos(band, q1p)

                # --- q path: dense 3x3 conv -> q2 band ---
                q2b = [band_p.tile([pn, BROWS * W], BF, tag=f"q2b{i}",
                                   name=f"q2b{i}_{band}")
                       for i, (p0, pn) in enumerate(CBQ)]
                for tt in range(BTILES):
                    for mi, (o0, on) in enumerate(CBQ):
                        ps = psw.tile([on, NT], F32, tag="psq",
                                      name=f"q2ps{mi}_{band}_{tt}")
                        ps_v = ps[:].rearrange("p (r c) -> p r c", r=RT)
                        _conv_taps(
                            nc, ps_v,
                            lambda dy, dx, k, o0=o0, on=on:
                                w3T[k][:, (dy * 3 + dx) * C + o0:
                                       (dy * 3 + dx) * C + o0 + on],
                            lambda k: q1p[k][:].rearrange("p (r c) -> p r c", c=W),
                            RT * tt, 2)
                        _copy(nc, 1 - mi, q2b[mi][:, bass.ts(tt, NT)], ps[:])

                # --- kv pointwise ---
                kv1pk = [band_p.tile([96, PB * W], BF, tag=f"kv1pk{j}",
                                     name=f"kv1pk{j}_{band}") for j in range(2)]
                kv1pv = [band_p.tile([96, PB * W], BF, tag=f"kv1pv{j}",
                                     name=f"kv1pv{j}_{band}") for j in range(2)]
                pw_band(band, "xb", kvwT,
                        [(0, 96, 0), (96, 96, 1), (192, 96, 2), (288, 96, 3)],
                        kv1pk + kv1pv, "x")
                zero_halos(band, kv1pk + kv1pv)

                # --- batched q transpose for the whole band ---
                qTh = []
                for hb in range(2):
                    t = tpose.tile([128, BROWS // 2, C], BF, tag="qTb",
                                   name=f"qTb_{band}_{hb}", bufs=2)
                    hsl = bass.ds(hb * (BROWS // 2) * W, (BROWS // 2) * W)
                    nc.scalar.dma_start_transpose(t[:, :, 0:128], q2b[0][:, hsl])
                    nc.scalar.dma_start_transpose(t[:, :, 128:192], q2b[1][:, hsl])
                    qTh.append(t)

                # --- dw-k + per-tile k transpose + gram ---
                for tt in range(BTILES):
                    ktile = sm.tile([96, 2 * NT], BF, tag="ktile",
                                    name=f"ktile_{band}_{tt}", bufs=4)
                    for j in range(2):
                        dst = ktile[:, j * NT:(j + 1) * NT]
                        eng = dw_engine(band, True, tt, j)
                        if eng == "PE":
                            ps = psw.tile([96, NT], F32, tag="psk",
                                          name=f"dwkps{j}_{band}_{tt}")
                            ps_v = ps[:].rearrange("p (r c) -> p r c", r=RT)
                            _conv_taps(
                                nc, ps_v,
                                lambda dy, dx, k, j=j: dwdK[:, (j * 9 + dy * 3 + dx) * 96:
                                                            (j * 9 + dy * 3 + dx) * 96 + 96],
                                lambda k, j=j: kv1pk[j][:].rearrange("p (r c) -> p r c", c=W),
                                RT * tt, 1)
                            _copy(nc, j, dst, ps[:])
                        else:
                            _dw_taps_vec(
                                nc, nc.vector if eng == "DVE" else nc.gpsimd, dst,
                                lambda t, j=j: dwsK[:, j * 9 + t:j * 9 + t + 1],
                                kv1pk[j][:].rearrange("p (r c) -> p r c", c=W),
                                RT * tt, stg, f"k{band}_{tt}_{j}")
                    kTt = tpose.tile([128, RT, C], BF, tag="kTt",
                                     name=f"kTt_{band}_{tt}", bufs=2)
                    nc.scalar.dma_start_transpose(kTt[:, :, 0:96], ktile[:, 0:NT])
                    nc.scalar.dma_start_transpose(kTt[:, :, 96:192], ktile[:, NT:])
                    for ch in range(RT):
                        st = (chunk_no[0] == 0)
                        sp = (chunk_no[0] == n_chunk_total - 1)
                        for g in range(2):
                            qs = qTh[(tt * RT + ch) // (BROWS // 2)][:, (tt * RT + ch) % (BROWS // 2), 96 * g:96 * g + 96]
                            ks = kTt[:, ch, 96 * g:96 * g + 96]
                            nc.tensor.matmul(gram_slot("qq", g), qs, qs, start=st,
                                             stop=sp, skip_group_check=True)
                            nc.tensor.matmul(gram_slot("qk", g), qs, ks, start=st,
                                             stop=sp, skip_group_check=True)
                            nc.tensor.matmul(gram_slot("kk", g), ks, ks, start=st,
                                             stop=sp, skip_group_check=True)
                        chunk_no[0] += 1

                # --- dw-v -> vt ---
                for tt in range(BTILES):
                    ti = BTILES * band + tt
                    for j in range(2):
                        dst = vt[j][:, bass.ts(ti, NT)]
                        eng = dw_engine(band, False, tt, j)
                        if eng == "PE":
                            ps = psw.tile([96, NT], F32, tag="psv",
                                          name=f"dwvps{j}_{band}_{tt}")
                            ps_v = ps[:].rearrange("p (r c) -> p r c", r=RT)
                            _conv_taps(
                                nc, ps_v,
                                lambda dy, dx, k, j=j: dwdV[:, (j * 9 + dy * 3 + dx) * 96:
                                                            (j * 9 + dy * 3 + dx) * 96 + 96],
                                lambda k, j=j: kv1pv[j][:].rearrange("p (r c) -> p r c", c=W),
                                RT * tt, 1)
                            nc.vector.tensor_copy(dst, ps[:])
                        elif eng == "ACTPOOL":
                            _dw_taps_actpool(
                                nc, dst,
                                lambda t, j=j: dwsV[:, j * 9 + t:j * 9 + t + 1],
                                kv1pv[j][:].rearrange("p (r c) -> p r c", c=W),
                                RT * tt, stg, f"v{band}_{tt}_{j}")
                        else:
                            _dw_taps_vec(
                                nc, nc.vector, dst,
                                lambda t, j=j: dwsV[:, j * 9 + t:j * 9 + t + 1],
                                kv1pv[j][:].rearrange("p (r c) -> p r c", c=W),
                                RT * tt, stg, f"v{band}_{tt}_{j}")

            # ---- attention math on [96,96] tiles ----
            g0s = sm.tile([96, 480], F32, tag="g0s", name="g0s")
            g1s = sm.tile([96, 96], F32, tag="g1s", name="g1s")
            nc.vector.tensor_copy(g0s[:], gbank0[:])
            nc.vector.tensor_copy(g1s[:], gbank1[:])
            def gram_s(kind, g):
                idx = {("qq", 0): 0, ("qk", 0): 1, ("kk", 0): 2,
                       ("qq", 1): 3, ("qk", 1): 4}.get((kind, g))
                return g1s[:, :] if idx is None else g0s[:, idx * 96:(idx + 1) * 96]

            wfT = []
            for g in range(2):
                def invnorm(kind, tag):
                    m = sm.tile([96, 96], F32, tag="inmul", name=f"inmul{tag}")
                    nc.vector.tensor_tensor(m[:], gram_s(kind, g), eye96[:],
                                            op=mybir.AluOpType.mult)
                    d = sm.tile([96, 1], F32, tag="ind", name=f"ind{tag}")
                    nc.vector.tensor_reduce(d[:], m[:], axis=mybir.AxisListType.X,
                                            op=mybir.AluOpType.add)
                    nc.scalar.sqrt(d[:], d[:])
                    nc.vector.tensor_scalar_max(d[:], d[:], EPS)
                    iv = sm.tile([96, 1], F32, tag=f"inv{tag}", name=f"inv{tag}")
                    nc.vector.reciprocal(iv[:], d[:])
                    return iv
                invq = invnorm("qq", f"q{g}")
                invk = invnorm("kk", f"k{g}")
                nc.vector.tensor_tensor(invq[:], invq[:], temp96[:, g:g + 1],
                                        op=mybir.AluOpType.mult)
                ikrow = sm.tile([1, 96], F32, tag="ikrow", name=f"ikrow{g}")
                nc.sync.dma_start(ikrow[:], invk[:].rearrange("a b -> b a"))
                ikrow_bf = sm.tile([1, 96], BF, tag="ikrowbf", name=f"ikrowbf{g}")
                nc.vector.tensor_copy(ikrow_bf[:], ikrow[:])
                ones_bf = sm.tile([1, 96], BF, tag="onesbf", name=f"onesbf{g}")
                nc.vector.memset(ones_bf[:], 1.0)
                bc = psw.tile([96, 96], F32, tag="psq", name=f"bcast{g}")
                nc.tensor.matmul(bc[:], ones_bf[:], ikrow_bf[:], start=True, stop=True)
                L = sm.tile([96, 96], F32, tag="L", name=f"L{g}")
                nc.vector.tensor_scalar_mul(L[:], gram_s("qk", g), invq[:])
                nc.vector.tensor_tensor(L[:], L[:], bc[:], op=mybir.AluOpType.mult)
                nc.vector.tensor_tensor(L[:], L[:], amask[:], op=mybir.AluOpType.add)
                mx = sm.tile([96, 1], F32, tag="mx", name=f"mx{g}")
                nc.vector.tensor_reduce(mx[:], L[:], axis=mybir.AxisListType.X,
                                        op=mybir.AluOpType.max)
                nc.vector.tensor_scalar_sub(L[:], L[:], mx[:])
                sme = sm.tile([96, 1], F32, tag="sme", name=f"sme{g}")
                nc.scalar.activation(L[:], L[:], AF.Exp, accum_out=sme[:])
                rden = sm.tile([96, 1], F32, tag="rden", name=f"rden{g}")
                nc.vector.reciprocal(rden[:], sme[:])
                Abf = sm.tile([96, 96], BF, tag="Abf", name=f"Abf{g}")
                nc.vector.tensor_scalar_mul(Abf[:], L[:], rden[:])
                wps = psw.tile([96, C], F32, tag="psq", name=f"wfps{g}")
                nc.tensor.matmul(wps[:], Abf[:], projT[g][:], start=True, stop=True)
                wf = sm.tile([96, C], BF, tag=f"wfT{g}", name=f"wfT{g}")
                nc.vector.tensor_copy(wf[:], wps[:])
                wfT.append(wf)

            # ---- out = wfT.T @ v ----
            for ti in range(NTILES):
                sl = bass.ts(ti, NT)
                for mi, (o0, on) in enumerate(CBQ):
                    ps = psw.tile([on, NT], F32, tag="psq", name=f"ops{mi}_{ti}")
                    for g in range(2):
                        nc.tensor.matmul(ps[:], wfT[g][:, o0:o0 + on], vt[g][:, sl],
                                         start=(g == 0), stop=(g == 1))
                    ot = stg.tile([on, NT], F32, tag=f"st{mi}", name=f"ostg{mi}_{ti}")
                    _copy(nc, mi, ot[:], ps[:])
                    nc.sync.dma_start(out_ap[o0:o0 + on, sl], ot[:])

    nc.compile()
    _NC_CACHE['nc'] = nc
    return nc


def _host_weights(temperature, kv_w, kv_dw_w, q_w, q_dw_w, proj_w):
    bf = ml_dtypes.bfloat16
    qwT = np.ascontiguousarray(q_w[:, :, 0, 0].T).astype(bf)
    w3T = np.ascontiguousarray(
        q_dw_w.transpose(1, 2, 3, 0).reshape(C, 9 * C)).astype(bf)
    kvwT = np.ascontiguousarray(kv_w[:, :, 0, 0].T).astype(bf)

    def diag_blocks(dw):
        out = np.zeros((96, 2 * 9 * 96), np.float32)
        for j in range(2):
            for t in range(9):
                w = dw[96 * j:96 * j + 96, 0, t // 3, t % 3]
                out[:, (j * 9 + t) * 96:(j * 9 + t + 1) * 96] = np.diag(w)
        return out.astype(bf)
    dwdK = diag_blocks(kv_dw_w[0:C])
    dwdV = diag_blocks(kv_dw_w[C:2 * C])
    def col_blocks(dw):
        out = np.zeros((96, 18), np.float32)
        for j in range(2):
            for t in range(9):
                out[:, j * 9 + t] = dw[96 * j:96 * j + 96, 0, t // 3, t % 3]
        return out
    dwsK = col_blocks(kv_dw_w[0:C])
    dwsV = col_blocks(kv_dw_w[C:2 * C])
    projT = np.ascontiguousarray(proj_w[:, :, 0, 0].T).astype(bf)
    tvec = np.asarray(temperature).reshape(HEADS)
    temp96 = np.ascontiguousarray(
        np.repeat(tvec, HD).reshape(2, 96).T.astype(np.float32))
    eye96 = np.eye(96, dtype=np.float32)
    amask = np.full((96, 96), -1e30, np.float32)
    for hh in range(4):
        amask[hh * HD:(hh + 1) * HD, hh * HD:(hh + 1) * HD] = 0.0
    return dict(qwT=qwT, w3T=w3T, kvwT=kvwT, dwdK=dwdK, dwdV=dwdV,
                dwsK=dwsK, dwsV=dwsV, projT=projT,
                temp96=temp96, eye96=eye96, amask=amask)


def kernel(x, y, temperature, kv_w, kv_dw_w, q_w, q_dw_w, proj_w, _trace=False):
    x = np.asarray(x, np.float32); y = np.asarray(y, np.float32)
    nc = build_bass()
    wts = _host_weights(np.asarray(temperature), np.asarray(kv_w),
                        np.asarray(kv_dw_w), np.asarray(q_w),
                        np.asarray(q_dw_w), np.asarray(proj_w))
    in_maps = []
    for b in range(B):
        m = dict(wts)
        m["xb"] = np.ascontiguousarray(x[b].reshape(C, HW))
        m["yb"] = np.ascontiguousarray(y[b].reshape(C, HW))
        in_maps.append(m)
    kw = dict(trace=True) if _trace else {}
    res = bass_utils.run_bass_kernel_spmd(nc, in_maps, core_ids=list(range(B)), **kw)
    out = np.stack([res.results[b]["outb"].reshape(C, H, W) for b in range(B)])
    kernel._last_results = res
    return out
